# revision 1
# baseline (speedup 1.0000x reference)
"""Trainium2 Bass kernel for the 3-layer AR GRU (nn_AR_RNN_GRU).

Strategy
--------
The time recurrence is strictly sequential (127 dependent steps x 3 layers),
and cross-core exchange on this part costs more than it saves (ncfw collective
floor ~5-9us vs ~3us of per-layer compute; remote SBUF-to-SBUF DMA is not
available under this runtime).  So the whole recurrence runs on ONE core with
the full batch of 64, organized to keep the PE and the vector engines busy:

 * "Folded" layout: a [64, 768] activation lives as [128, 384] in SBUF --
   batch on partitions 0-63 for units 0-383 and partitions 64-127 for units
   384-767.  All elementwise gate math then uses the full 128 lanes, and each
   weight matrix streams as two 1152-column halves through the two PE
   column-group pairs concurrently (tile_position via psum base partition).
 * Weights and matmul stationaries live in SBUF as fp16 (fp32 does not fit
   in SBUF; fp16 keeps the 127-step compounded error ~1e-2 where bf16 gave
   ~8e-2); the folded recurrent state and all PSUM accumulation stay fp32.
 * Gate pre-activations: psum_zr accumulates x@Wx + h@Wh for the z,r gates
   (the add is free in PSUM); the candidate keeps xh and hh separate so that
   hc = tanh(xh + r*hh) matches the reset_after GRU cell.
 * The AR feedback (normalize + dense) is folded into one effective matrix:
   gx0 = p2 @ (Wd @ (Wx0/std)) + beff, which removes the dense+normalize from
   the critical path; the actual prediction p2 @ Wd + bd is computed off-path.
 * h is re-transposed each layer via 3 PE-transposes of [128,128] tiles; the
   transposed tiles double as the lhsT (stationary) for the next matmuls and
   for the dense readout.
Biases are applied exactly via an extra "ones" K-chunk whose rhs row 0 holds
the bias vector -- emitted only when the bias is nonzero (in this problem all
bi/br/bd are zero; beff is nonzero and always emitted).
"""

import os
import sys

import numpy as np

try:
    import concourse.bass as bass  # noqa: F401
except ImportError:  # grading env fallback
    sys.path.insert(0, "/opt/trn_rl_repo")

import ml_dtypes

import concourse.bass as bass
import concourse.mybir as mybir
import concourse.tile as tile
from concourse import bacc
from concourse.bass_utils import run_bass_kernel_spmd
from concourse.masks import make_identity

BF16 = np.float16

B = 64  # batch
D = 512  # data dim
U = 768  # GRU units
G = 3 * U  # gate columns
HALF = U // 2  # 384

T_IN = int(os.environ.get("GRU_TIN", "64"))
T_OUT = int(os.environ.get("GRU_TOUT", "64"))

# column permutation: [z_lo r_lo h_lo | z_hi r_hi h_hi], each block 384 wide
_PERM = np.concatenate(
    [
        np.arange(0, HALF),
        np.arange(U, U + HALF),
        np.arange(2 * U, 2 * U + HALF),
        np.arange(HALF, U),
        np.arange(U + HALF, 2 * U),
        np.arange(2 * U + HALF, G),
    ]
)


def _prep_weight(w, bias):
    """[K, 2304] fp32 (+bias [2304]) -> ([n_chunks, 128, 2304] bf16, has_bias)."""
    k = w.shape[0]
    assert k % 128 == 0
    wp = np.ascontiguousarray(w[:, _PERM]).reshape(k // 128, 128, G)
    has_bias = bias is not None and float(np.abs(bias).max()) > 0.0
    if has_bias:
        bc = np.zeros((1, 128, G), np.float32)
        bc[0, 0, :] = bias[_PERM]
        wp = np.concatenate([wp, bc], axis=0)
    return wp.astype(BF16), has_bias


def _fold(a):
    """[64, 768] -> folded [128, 384]."""
    return np.concatenate([a[:, :HALF], a[:, HALF:]], axis=0)


def _build(n_warm, n_ar, bias_flags):
    """Build the Bass program. bias_flags: dict name->bool for extra chunks."""
    nc = bacc.Bacc(num_devices=1, name="gru_ar")
    f32, bf16 = mybir.dt.float32, mybir.dt.float16
    n_steps = n_warm + n_ar

    # ---- DRAM I/O ----
    def wchunks(name, kc):
        return nc.dram_tensor(name, [kc * 128, G], bf16, kind="ExternalInput"), kc

    wx0, wx0_c = wchunks("wx0", 4 + bias_flags["bi0"])
    weff, weff_c = wchunks("weff", 6 + bias_flags["beff"])
    wx = [None, *(wchunks(f"wx{j}", 6 + bias_flags[f"bi{j}"]) for j in (1, 2))]
    wh = [wchunks(f"wh{j}", 6 + bias_flags[f"br{j}"]) for j in (0, 1, 2)]
    wd_c = 6 + bias_flags["bd"]
    wd = nc.dram_tensor("wd", [wd_c * 128, D], bf16, kind="ExternalInput")
    xt = nc.dram_tensor("xt", [n_warm * 4 * 128, B], bf16, kind="ExternalInput")
    h0f = nc.dram_tensor("h0f", [3 * 128, HALF], f32, kind="ExternalInput")
    h0t = nc.dram_tensor("h0t", [3 * 128, 3 * 128], bf16, kind="ExternalInput")
    ones = nc.dram_tensor("ones", [128, 128], bf16, kind="ExternalInput")
    out = nc.dram_tensor("out", [B, n_ar + 1, D], f32, kind="ExternalOutput")
    debug = os.environ.get("GRU_DEBUG", "") == "1"
    if debug:
        dbg_zr = nc.dram_tensor("dbg_zr", [128, 2 * HALF], f32, kind="ExternalOutput")
        dbg_hf = nc.dram_tensor("dbg_hf", [128, HALF], f32, kind="ExternalOutput")
        dbg_xt = nc.dram_tensor("dbg_xt", [128, 4 * B], f32, kind="ExternalOutput")
        dbg_ht = nc.dram_tensor("dbg_ht", [128, 3 * 128], f32, kind="ExternalOutput")

    with tile.TileContext(nc) as tc:
        with (
            tc.tile_pool(name="wpool", bufs=1) as wpool,
            tc.tile_pool(name="state", bufs=1) as spool,
            tc.tile_pool(name="work", bufs=2) as work,
            tc.tile_pool(name="workb", bufs=2) as workb,
            tc.tile_pool(name="xs", bufs=3) as xs,
            tc.tile_pool(name="pzr", bufs=2, space="PSUM") as pzr,
            tc.tile_pool(name="ph", bufs=2, space="PSUM") as ph,
            tc.tile_pool(name="paux", bufs=2, space="PSUM") as paux,
        ):
            # ---- load weights / constants ----
            def load_w(dram, kc, ncols):
                t = wpool.tile([128, kc * ncols], bf16, tag=dram.name)
                for c in range(kc):
                    nc.sync.dma_start(
                        t[:, c * ncols : (c + 1) * ncols],
                        dram[c * 128 : (c + 1) * 128, :],
                    )
                return t

            # Wx0 (warmup) and Weff (AR) share one SBUF slot; Weff is DMA'd
            # over Wx0 after the last warmup gx0 matmul (Tile orders the WAR).
            # small step-0 state/constants first, then weights in the order
            # the first step consumes them (wx0, wh0, wx1, wh1, ...), so the
            # PE can start while the rest of the ~21 MB is still in flight
            ones_t = wpool.tile([128, 128], bf16, tag="ones")
            nc.sync.dma_start(ones_t[:], ones[:])
            ident = wpool.tile([128, 128], f32, tag="ident")
            make_identity(nc, ident[:])
            ident16 = wpool.tile([128, 128], bf16, tag="ident16")
            nc.vector.tensor_copy(ident16[:], ident[:])
            hF = []
            hT = []
            for j in range(3):
                f = spool.tile([128, HALF], f32, tag=f"hF{j}")
                nc.sync.dma_start(f[:], h0f[j * 128 : (j + 1) * 128, :])
                hF.append(f)
                t = spool.tile([128, 3 * 128], bf16, tag=f"hT{j}")
                nc.sync.dma_start(t[:], h0t[j * 128 : (j + 1) * 128, :])
                hT.append(t)

            nshare = max(wx0_c, weff_c)
            wshare_t = wpool.tile([128, nshare * G], bf16, tag="wx0weff")
            for c in range(wx0_c):
                nc.sync.dma_start(
                    wshare_t[:, c * G : (c + 1) * G], wx0[c * 128 : (c + 1) * 128, :]
                )
            wx0_t = wshare_t
            weff_t = wshare_t
            wh_t = [None, None, None]
            wx_t = [None, None, None]
            wh_t[0] = load_w(wh[0][0], wh[0][1], G)
            # prefetch the first two warmup x tiles ahead of the bulk weights
            xpre = {}
            for tt in range(min(2, n_warm)):
                xtile = xs.tile([128, 4 * B], bf16, tag="xt")
                for c in range(4):
                    nc.sync.dma_start(
                        xtile[:, c * B : (c + 1) * B],
                        xt[tt * 512 + c * 128 : tt * 512 + (c + 1) * 128, :],
                    )
                xpre[tt] = xtile
            wx_t[1] = load_w(wx[1][0], wx[1][1], G)
            wh_t[1] = load_w(wh[1][0], wh[1][1], G)
            wx_t[2] = load_w(wx[2][0], wx[2][1], G)
            wh_t[2] = load_w(wh[2][0], wh[2][1], G)
            wd_t = load_w(wd, wd_c, D)

            def ht_slice(t, c):
                # K-chunk c (0..5) of the transposed folded state tile set
                if c < 3:
                    return t[:, c * 128 : c * 128 + 64]
                return t[:, (c - 3) * 128 + 64 : (c - 3) * 128 + 128]

            # ---- one recurrent step ----
            def stream(psum_zr, psum_h, w_t, kc, lhsT_fn, first_zr, last_zr):
                """Emit the matmuls of one weight stream (both folded halves).
                6-chunk streams go in order 0,3,1,4,2,5 so that each transposed
                state tile T_c unlocks its two K-chunks (c, c+3) as it lands."""
                order = [0, 3, 1, 4, 2, 5] + list(range(6, kc)) if kc >= 6 else list(range(kc))
                for ci, c in enumerate(order):
                    lhsT = lhsT_fn(c)
                    for h_ix in range(2):
                        base = 64 * h_ix
                        off = h_ix * (G // 2)
                        first = first_zr and ci == 0
                        last = last_zr and ci == kc - 1
                        nc.tensor.matmul(
                            psum_zr[base : base + 64, 0:512],
                            lhsT,
                            w_t[:, c * G + off : c * G + off + 512],
                            start=first,
                            stop=last,
                        )
                        nc.tensor.matmul(
                            psum_zr[base : base + 64, 512:768],
                            lhsT,
                            w_t[:, c * G + off + 512 : c * G + off + 768],
                            start=first,
                            stop=last,
                        )
                        nc.tensor.matmul(
                            psum_h[base : base + 64, 0:HALF],
                            lhsT,
                            w_t[:, c * G + off + 768 : c * G + off + 1152],
                            start=ci == 0,
                            stop=ci == kc - 1,
                        )

            def gru_layer(j, gx_w, gx_kc, gx_lhsT_fn):
                psum_zr = pzr.tile([128, 2 * HALF], f32, tag="zr")
                psum_xh = ph.tile([128, HALF], f32, tag="xh")
                psum_hh = paux.tile([128, HALF], f32, tag="aux")
                # recurrent stream first (inputs available earliest)
                wh_kc = wh[j][1]

                def gh_lhsT(c):
                    return ones_t[:, 0:64] if c >= 6 else ht_slice(hT[j], c)

                stream(psum_zr, psum_hh, wh_t[j], wh_kc, gh_lhsT, True, False)
                stream(psum_zr, psum_xh, gx_w, gx_kc, gx_lhsT_fn, False, True)

                # gates (folded [128, *])
                zr = work.tile([128, 2 * HALF], f32, tag="zr_s")
                # r first: it gates the critical path (r*hh); z can lag
                nc.scalar.activation(
                    zr[:, HALF : 2 * HALF],
                    psum_zr[:, HALF : 2 * HALF],
                    mybir.ActivationFunctionType.Sigmoid,
                )
                t1 = workb.tile([128, HALF], f32, tag="t1")
                nc.vector.tensor_mul(t1[:], zr[:, HALF : 2 * HALF], psum_hh[:])
                nc.scalar.activation(
                    zr[:, 0:HALF],
                    psum_zr[:, 0:HALF],
                    mybir.ActivationFunctionType.Sigmoid,
                )
                if debug and j == 0:
                    nc.sync.dma_start(dbg_zr[:], zr[:])
                nc.vector.tensor_add(t1[:], t1[:], psum_xh[:])
                hc = workb.tile([128, HALF], f32, tag="hc")
                nc.scalar.activation(hc[:], t1[:], mybir.ActivationFunctionType.Tanh)
                # h_new = hc + z*(h_prev - hc)
                d = workb.tile([128, HALF], f32, tag="d")
                nc.vector.tensor_sub(d[:], hF[j][:], hc[:])
                nc.vector.tensor_mul(d[:], zr[:, 0:HALF], d[:])
                nc.vector.tensor_add(hF[j][:], d[:], hc[:])
                # duplicate fp16 state write: lets the transposes run at
                # 1 cyc/row (fp16) with bit-identical hT (same rounding point)
                h16 = workb.tile([128, HALF], bf16, tag="h16")
                nc.vector.tensor_add(h16[:], d[:], hc[:])
                # re-transpose the folded state for the next matmuls
                for c in range(3):
                    ptr = ph.tile([128, 128], bf16, tag="xh")
                    nc.tensor.transpose(
                        ptr[:], h16[:, c * 128 : (c + 1) * 128], ident16[:]
                    )
                    # alternate engines so psum->sbuf copies run in parallel
                    if c == 1:
                        nc.vector.tensor_copy(hT[j][:, c * 128 : (c + 1) * 128], ptr[:])
                    else:
                        nc.scalar.copy(hT[j][:, c * 128 : (c + 1) * 128], ptr[:])
                if debug and j == 0:
                    nc.sync.dma_start(dbg_hf[:], hF[j][:])
                    tconv = workb.tile([128, 3 * 128], f32, tag="tconv")
                    nc.vector.tensor_copy(tconv[:], hT[j][:])
                    nc.sync.dma_start(dbg_ht[:], tconv[:])

            for t in range(n_steps):
                warm = t < n_warm
                if warm:
                    if t in xpre:
                        xtile = xpre.pop(t)
                    else:
                        xtile = xs.tile([128, 4 * B], bf16, tag="xt")
                        for c in range(4):
                            nc.sync.dma_start(
                                xtile[:, c * B : (c + 1) * B],
                                xt[t * 512 + c * 128 : t * 512 + (c + 1) * 128, :],
                            )

                    def gx0_lhsT(c, _x=xtile):
                        return (
                            ones_t[:, 0:64]
                            if c >= 4
                            else _x[:, c * B : c * B + B]
                        )

                    if debug and t == 0:
                        xconv = workb.tile([128, 4 * B], f32, tag="xconv")
                        nc.vector.tensor_copy(xconv[:], xtile[:])
                        nc.sync.dma_start(dbg_xt[:], xconv[:])

                    gru_layer(0, wx0_t, wx0_c, gx0_lhsT)
                    if t == n_warm - 1:
                        # overwrite the shared slot with Weff for the AR phase
                        for c in range(weff_c):
                            nc.sync.dma_start(
                                wshare_t[:, c * G : (c + 1) * G],
                                weff[c * 128 : (c + 1) * 128, :],
                            )
                else:

                    def gxar_lhsT(c):
                        return ones_t[:, 0:64] if c >= 6 else ht_slice(hT[2], c)

                    gru_layer(0, weff_t, weff_c, gxar_lhsT)

                for j in (1, 2):

                    def gx_lhsT(c, _j=j):
                        return ones_t[:, 0:64] if c >= 6 else ht_slice(hT[_j - 1], c)

                    gru_layer(j, wx_t[j], wx[j][1], gx_lhsT)

                # dense readout: pred = p2 @ Wd (+bd), off the critical path
                if t >= n_warm - 1:
                    prd = paux.tile([64, 512], f32, tag="aux")
                    for c in range(wd_c):
                        lhsT = ones_t[:, 0:64] if c >= 6 else ht_slice(hT[2], c)
                        nc.tensor.matmul(
                            prd[0:64, :],
                            lhsT,
                            wd_t[:, c * D : (c + 1) * D],
                            start=c == 0,
                            stop=c == wd_c - 1,
                        )
                    prs = workb.tile([64, 512], f32, tag="pred")
                    nc.vector.tensor_copy(prs[:], prd[:])
                    nc.sync.dma_start(out[:, t - (n_warm - 1), :], prs[:])
    nc.finalize()
    return nc


def kernel(**inputs):
    x = np.asarray(inputs["inputs"], np.float32)
    n_warm, n_ar = T_IN, T_OUT - 1
    x = x[:, :n_warm, :]

    mean = np.asarray(inputs["mean"], np.float32)[0]
    std = np.asarray(inputs["std"], np.float32)[0]
    wd_m = np.asarray(inputs["Wd"], np.float32)
    bd = np.asarray(inputs["bd"], np.float32)
    w1 = np.asarray(inputs["Wx0"], np.float32) / std[:, None]
    weff_m = wd_m @ w1
    beff = (bd - mean) @ w1 + np.asarray(inputs["bi0"], np.float32)

    bias_flags = {}
    wx0_a, bias_flags["bi0"] = _prep_weight(
        np.asarray(inputs["Wx0"], np.float32), np.asarray(inputs["bi0"], np.float32)
    )
    weff_a, has_beff = _prep_weight(weff_m, beff)
    bias_flags["beff"] = has_beff
    wx_a = {}
    wh_a = {}
    for j in range(3):
        if j > 0:
            wx_a[j], bias_flags[f"bi{j}"] = _prep_weight(
                np.asarray(inputs[f"Wx{j}"], np.float32),
                np.asarray(inputs[f"bi{j}"], np.float32),
            )
        wh_a[j], bias_flags[f"br{j}"] = _prep_weight(
            np.asarray(inputs[f"Wh{j}"], np.float32),
            np.asarray(inputs[f"br{j}"], np.float32),
        )
    # dense readout chunks (no column permutation)
    wd_p = wd_m.reshape(6, 128, D)
    bias_flags["bd"] = float(np.abs(bd).max()) > 0.0
    if bias_flags["bd"]:
        bc = np.zeros((1, 128, D), np.float32)
        bc[0, 0, :] = bd
        wd_p = np.concatenate([wd_p, bc], axis=0)
    wd_a = wd_p.astype(BF16)

    # warmup inputs, transposed per step: [T, D, B] -> [T*4*128, B]
    xt_a = np.ascontiguousarray(x.transpose(1, 2, 0)).reshape(n_warm * 512, B)
    xt_a = xt_a.astype(BF16)

    h0f_l = []
    h0t_l = []
    for j in range(3):
        h0 = np.tile(np.asarray(inputs[f"h0_{j}"], np.float32), (B, 1))
        f = _fold(h0)  # [128, 384]
        h0f_l.append(f)
        tchunks = [f[:, c * 128 : (c + 1) * 128].T for c in range(3)]
        h0t_l.append(np.concatenate(tchunks, axis=1))
    h0f_a = np.concatenate(h0f_l, axis=0).astype(np.float32)
    h0t_a = np.concatenate(h0t_l, axis=0).astype(BF16)

    ones_a = np.zeros((128, 128), np.float32)
    ones_a[0, :] = 1.0
    ones_a = ones_a.astype(BF16)

    nc = _build(n_warm, n_ar, bias_flags)
    in_map = {
        "wx0": wx0_a.reshape(-1, G),
        "weff": weff_a.reshape(-1, G),
        "wx1": wx_a[1].reshape(-1, G),
        "wx2": wx_a[2].reshape(-1, G),
        "wh0": wh_a[0].reshape(-1, G),
        "wh1": wh_a[1].reshape(-1, G),
        "wh2": wh_a[2].reshape(-1, G),
        "wd": wd_a.reshape(-1, D),
        "xt": xt_a,
        "h0f": h0f_a,
        "h0t": h0t_a,
        "ones": ones_a,
    }
    res = run_bass_kernel_spmd(
        nc,
        [in_map],
        core_ids=[0],
        trace=os.environ.get("GRU_TRACE", "") == "1",
    )
    kernel._last = res
    kernel._last_nc = nc
    return np.asarray(res.results[0]["out"], np.float32)


if __name__ == "__main__":
    rng = np.random.RandomState(0)
    print("smoke build only")



# revision 8
# speedup vs baseline: 1.8587x; 1.8587x over previous
"""Trainium2 Bass kernel for the 3-layer AR GRU (nn_AR_RNN_GRU).

Strategy
--------
The time recurrence is strictly sequential (127 dependent steps x 3 layers),
and cross-core exchange costs more than it saves (collective floor ~15us in
the cost model; remote SBUF-to-SBUF DMA is not available under this runtime).
The whole recurrence runs on ONE core with the full batch of 64.

This version uses a *transposed, weight-stationary* formulation: matmul cost
on the PE is set by the moving-operand column count, so we make the weight
block [128K x 128M] the stationary operand and stream the transposed
activations h^T (64 batch columns) through it:

    gates^T[m-chunk, batch] += W[k-chunk, m-chunk]^T-as-lhsT @ h^T[k-chunk, :]

 * Each [128,128] weight block costs only 64 moving columns (vs 128-equiv in
   the batch-stationary layout) -> ~2x less PE time for the same FLOPs.
 * The state lives as h^T tiles [128 unit-chunk, 64 batch] (fp16 for matmul,
   fp32 for the state update), so layer outputs are ALREADY in the rhs layout
   of the next matmuls: no PE transposes at all.
 * Gate psums are [128 gate-chunk, 64] slices packed into [128, 384] banks
   (Z, R, XH, HH x 2 parities = 8 PSUM banks); gate math runs on the full
   [128, 384] spans on ACT (sigmoid/tanh, and 1-z via scale=-1) and DVE.
 * The AR feedback folds dense+normalize into one effective matrix:
   gx0 = h2 @ (Wd @ (Wx0/std)) + beff, removing the dense+normalize from the
   critical path; the real prediction h2 @ Wd + bd streams off-path and is
   written transposed to DRAM (the host un-transposes at the end).
 * Per-slice gx emission order [r, z, xh] shortens the gate-math critical
   chain after the last matmul so the PE never stalls on the state update.
Biases are applied via an extra "ones" K-chunk whose lhsT row 0 holds the
bias vector -- only beff is nonzero in this problem.
"""

import os
import sys

import numpy as np

try:
    import concourse.bass as bass  # noqa: F401
except ImportError:  # grading env fallback
    sys.path.insert(0, "/opt/trn_rl_repo")

import concourse.bass as bass
import concourse.mybir as mybir
import concourse.tile as tile
from concourse import bacc
from concourse.bass_utils import run_bass_kernel_spmd

F16 = np.float16

B = 64  # batch
D = 512  # data dim
U = 768  # GRU units
G = 3 * U  # gate columns (z|r|h)
KU = U // 128  # 6 K-chunks for a 768-row operand
MG = G // 128  # 18 M-chunks of gate columns
MD = D // 128  # 4 M-chunks of data columns

T_IN = int(os.environ.get("GRU_TIN", "64"))
T_OUT = int(os.environ.get("GRU_TOUT", "64"))

# m-chunk emission order: r slices first (they start the gate-math chain),
# then z, then the candidate (xh/hh) slices.
M_ORDER = list(range(6, 12)) + list(range(0, 6)) + list(range(12, 18))


def _prep_weight(w, bias):
    """[K, G] fp32 (+bias [G]) -> ([n_chunks*128, G] fp16, has_bias)."""
    k = w.shape[0]
    assert k % 128 == 0
    wp = w.reshape(k // 128, 128, G)
    has_bias = bias is not None and float(np.abs(bias).max()) > 0.0
    if has_bias:
        bc = np.zeros((1, 128, G), np.float32)
        bc[0, 0, :] = bias
        wp = np.concatenate([wp, bc], axis=0)
    return wp.astype(F16).reshape(-1, G), has_bias


def _build(n_warm, n_ar, bias_flags):
    nc = bacc.Bacc(num_devices=1, name="gru_ar_t")
    f32, f16 = mybir.dt.float32, mybir.dt.float16
    n_steps = n_warm + n_ar
    n_out = n_ar + 1

    # ---- DRAM I/O ----
    wx0_c = 4 + bias_flags["bi0"]
    weff_c = 6 + bias_flags["beff"]
    wx0 = nc.dram_tensor("wx0", [wx0_c * 128, G], f16, kind="ExternalInput")
    weff = nc.dram_tensor("weff", [weff_c * 128, G], f16, kind="ExternalInput")
    wx_c = [wx0_c, 6 + bias_flags["bi1"], 6 + bias_flags["bi2"]]
    wx = [None] + [
        nc.dram_tensor(f"wx{j}", [wx_c[j] * 128, G], f16, kind="ExternalInput")
        for j in (1, 2)
    ]
    wh_c = [6 + bias_flags[f"br{j}"] for j in range(3)]
    wh = [
        nc.dram_tensor(f"wh{j}", [wh_c[j] * 128, G], f16, kind="ExternalInput")
        for j in range(3)
    ]
    wd_c = 6 + bias_flags["bd"]
    wd = nc.dram_tensor("wd", [wd_c * 128, D], f16, kind="ExternalInput")
    xt = nc.dram_tensor("xt", [n_warm * 4 * 128, B], f16, kind="ExternalInput")
    h0f = nc.dram_tensor("h0f", [3 * 128, KU * B], f32, kind="ExternalInput")
    ones = nc.dram_tensor("ones", [128, B], f16, kind="ExternalInput")
    # transposed output: step s, D-chunk m at rows (s*4+m)*128
    out = nc.dram_tensor("out", [n_out * 4 * 128, B], f32, kind="ExternalOutput")

    with tile.TileContext(nc) as tc:
        with (
            tc.tile_pool(name="wpool", bufs=1) as wpool,
            tc.tile_pool(name="state", bufs=1) as spool,
            tc.tile_pool(name="work", bufs=2) as work,
            tc.tile_pool(name="workb", bufs=2) as workb,
            tc.tile_pool(name="xs", bufs=3) as xs,
            tc.tile_pool(name="pz", bufs=2, space="PSUM") as pzp,
            tc.tile_pool(name="pr", bufs=2, space="PSUM") as prp,
            tc.tile_pool(name="pxh", bufs=2, space="PSUM") as pxhp,
            tc.tile_pool(name="phh", bufs=2, space="PSUM") as phhp,
        ):

            def load_w(dram, kc, ncols, tag):
                t = wpool.tile([128, kc * ncols], f16, tag=tag)
                for c in range(kc):
                    nc.sync.dma_start(
                        t[:, c * ncols : (c + 1) * ncols],
                        dram[c * 128 : (c + 1) * 128, :],
                    )
                return t

            # small constants + state first, then weights in first-use order
            ones_t = wpool.tile([128, B], f16, tag="ones")
            nc.sync.dma_start(ones_t[:], ones[:])
            hF = []  # fp32 transposed state [128, KU*64]
            hT = []  # fp16 copy for matmuls
            for j in range(3):
                f = spool.tile([128, KU * B], f32, tag=f"hF{j}")
                nc.sync.dma_start(f[:], h0f[j * 128 : (j + 1) * 128, :])
                hF.append(f)
                t = spool.tile([128, KU * B], f16, tag=f"hT{j}")
                hT.append(t)
                nc.vector.tensor_copy(t[:], f[:])

            wh_t = [load_w(wh[0], wh_c[0], G, "wh0")]
            # prefetch the first two warmup x tiles ahead of the bulk weights
            xpre = {}
            for tt in range(min(2, n_warm)):
                xtile = xs.tile([128, 4 * B], f16, tag="xt")
                for c in range(4):
                    nc.sync.dma_start(
                        xtile[:, c * B : (c + 1) * B],
                        xt[tt * 512 + c * 128 : tt * 512 + (c + 1) * 128, :],
                    )
                xpre[tt] = xtile
            nshare = max(wx0_c, weff_c)
            wshare_t = wpool.tile([128, nshare * G], f16, tag="wx0weff")
            for c in range(wx0_c):
                nc.sync.dma_start(
                    wshare_t[:, c * G : (c + 1) * G], wx0[c * 128 : (c + 1) * 128, :]
                )
            wx_t = [wshare_t]
            wh_t.append(load_w(wh[1], wh_c[1], G, "wh1"))
            wx_t.append(load_w(wx[1], wx_c[1], G, "wx1"))
            wh_t.append(load_w(wh[2], wh_c[2], G, "wh2"))
            wx_t.append(load_w(wx[2], wx_c[2], G, "wx2"))
            wd_t = load_w(wd, wd_c, D, "wd")

            sig = mybir.ActivationFunctionType.Sigmoid
            tanh = mybir.ActivationFunctionType.Tanh

            def gru_layer(j, gx_w, gx_kc, gx_rhs_fn):
                pz = pzp.tile([128, 6 * B], f32, tag="z")
                pr = prp.tile([128, 6 * B], f32, tag="r")
                pxh = pxhp.tile([128, 6 * B], f32, tag="xh")
                phh = phhp.tile([128, 6 * B], f32, tag="hh")

                def gh_rhs(k, _j=j):
                    return (
                        ones_t[:, 0:B] if k >= 6 else hT[_j][:, k * B : (k + 1) * B]
                    )

                # start=True clears the WHOLE psum bank, so exactly one
                # start (the first matmul into each tile) and one stop (the
                # last) per tile; interleaved slices rely on has_written
                # bits for overwrite-on-first-touch.
                plan = []  # (dst, m, w_t, k, rhs_fn)
                for k in range(wh_c[j]):
                    for m in M_ORDER:
                        dst = pz if m < 6 else (pr if m < 12 else phh)
                        plan.append((dst, m, wh_t[j], k, gh_rhs))
                for k in range(gx_kc):
                    for m in M_ORDER:
                        dst = pz if m < 6 else (pr if m < 12 else pxh)
                        plan.append((dst, m, gx_w, k, gx_rhs_fn))
                first_of = {}
                last_of = {}
                for i, (dst, *_rest) in enumerate(plan):
                    if id(dst) not in first_of:
                        first_of[id(dst)] = i
                    last_of[id(dst)] = i
                for i, (dst, m, w_t, k, rhs_fn) in enumerate(plan):
                    c = m % 6
                    nc.tensor.matmul(
                        dst[:, c * B : (c + 1) * B],
                        w_t[:, k * G + m * 128 : k * G + (m + 1) * 128],
                        rhs_fn(k),
                        start=first_of[id(dst)] == i,
                        stop=last_of[id(dst)] == i,
                        skip_group_check=True,
                    )

                # gate math on [128, 384] spans
                rs = work.tile([128, 6 * B], f32, tag="rs")
                nc.scalar.activation(rs[:], pr[:], sig)
                zs = work.tile([128, 6 * B], f32, tag="zs")
                nc.scalar.activation(zs[:], pz[:], sig)
                omz = work.tile([128, 6 * B], f32, tag="omz")
                nc.scalar.activation(omz[:], pz[:], sig, scale=-1.0)
                t1 = workb.tile([128, 6 * B], f32, tag="t1")
                nc.vector.tensor_mul(t1[:], rs[:], phh[:])
                zh = workb.tile([128, 6 * B], f32, tag="zh")
                nc.vector.tensor_mul(zh[:], zs[:], hF[j][:])
                nc.vector.tensor_add(t1[:], t1[:], pxh[:])
                hc = workb.tile([128, 6 * B], f32, tag="hc")
                nc.scalar.activation(hc[:], t1[:], tanh)
                q = workb.tile([128, 6 * B], f32, tag="q")
                nc.vector.tensor_mul(q[:], omz[:], hc[:])
                # fp16 state first: it is the next matmul's input
                nc.vector.tensor_add(hT[j][:], zh[:], q[:])
                nc.vector.tensor_add(hF[j][:], zh[:], q[:])

            for t in range(n_steps):
                warm = t < n_warm
                if warm:
                    if t in xpre:
                        xtile = xpre.pop(t)
                    else:
                        xtile = xs.tile([128, 4 * B], f16, tag="xt")
                        for c in range(4):
                            nc.sync.dma_start(
                                xtile[:, c * B : (c + 1) * B],
                                xt[t * 512 + c * 128 : t * 512 + (c + 1) * 128, :],
                            )

                    def gx0_rhs(k, _x=xtile):
                        return (
                            ones_t[:, 0:B] if k >= 4 else _x[:, k * B : (k + 1) * B]
                        )

                    gru_layer(0, wshare_t, wx0_c, gx0_rhs)
                    if t == n_warm - 1:
                        # overwrite the shared slot with Weff for the AR phase
                        for c in range(weff_c):
                            nc.sync.dma_start(
                                wshare_t[:, c * G : (c + 1) * G],
                                weff[c * 128 : (c + 1) * 128, :],
                            )
                else:

                    def gxar_rhs(k):
                        return (
                            ones_t[:, 0:B] if k >= 6 else hT[2][:, k * B : (k + 1) * B]
                        )

                    gru_layer(0, wshare_t, weff_c, gxar_rhs)

                # dense readout of h2(t-1): emitted after layer 0 of step t so
                # the PE never waits on the h2 state update
                if t >= n_warm:
                    pp = pxhp.tile([128, 6 * B], f32, tag="xh")
                    for k in range(wd_c):
                        rhs = (
                            ones_t[:, 0:B]
                            if k >= 6
                            else hT[2][:, k * B : (k + 1) * B]
                        )
                        for m in range(MD):
                            nc.tensor.matmul(
                                pp[:, m * B : (m + 1) * B],
                                wd_t[:, k * D + m * 128 : k * D + (m + 1) * 128],
                                rhs,
                                start=k == 0 and m == 0,
                                stop=k == wd_c - 1 and m == MD - 1,
                                skip_group_check=True,
                            )
                    prs = workb.tile([128, MD * B], f32, tag="pred")
                    nc.vector.tensor_copy(prs[:], pp[:, 0 : MD * B])
                    s = t - n_warm
                    for m in range(MD):
                        nc.sync.dma_start(
                            out[(s * 4 + m) * 128 : (s * 4 + m + 1) * 128, :],
                            prs[:, m * B : (m + 1) * B],
                        )

                for j in (1, 2):

                    def gx_rhs(k, _j=j):
                        return (
                            ones_t[:, 0:B]
                            if k >= 6
                            else hT[_j - 1][:, k * B : (k + 1) * B]
                        )

                    gru_layer(j, wx_t[j], wx_c[j], gx_rhs)

            # final prediction (from h2 of the last step)
            pp = pxhp.tile([128, 6 * B], f32, tag="xh")
            for k in range(wd_c):
                rhs = ones_t[:, 0:B] if k >= 6 else hT[2][:, k * B : (k + 1) * B]
                for m in range(MD):
                    nc.tensor.matmul(
                        pp[:, m * B : (m + 1) * B],
                        wd_t[:, k * D + m * 128 : k * D + (m + 1) * 128],
                        rhs,
                        start=k == 0 and m == 0,
                        stop=k == wd_c - 1 and m == MD - 1,
                        skip_group_check=True,
                    )
            prs = workb.tile([128, MD * B], f32, tag="pred")
            nc.vector.tensor_copy(prs[:], pp[:, 0 : MD * B])
            for m in range(MD):
                nc.sync.dma_start(
                    out[(n_ar * 4 + m) * 128 : (n_ar * 4 + m + 1) * 128, :],
                    prs[:, m * B : (m + 1) * B],
                )
    nc.finalize()
    return nc


def kernel(**inputs):
    x = np.asarray(inputs["inputs"], np.float32)
    n_warm, n_ar = T_IN, T_OUT - 1
    x = x[:, :n_warm, :]

    mean = np.asarray(inputs["mean"], np.float32)[0]
    std = np.asarray(inputs["std"], np.float32)[0]
    wd_m = np.asarray(inputs["Wd"], np.float32)
    bd = np.asarray(inputs["bd"], np.float32)
    w1 = np.asarray(inputs["Wx0"], np.float32) / std[:, None]
    weff_m = wd_m @ w1
    beff = (bd - mean) @ w1 + np.asarray(inputs["bi0"], np.float32)

    bias_flags = {}
    wx0_a, bias_flags["bi0"] = _prep_weight(
        np.asarray(inputs["Wx0"], np.float32), np.asarray(inputs["bi0"], np.float32)
    )
    weff_a, bias_flags["beff"] = _prep_weight(weff_m, beff)
    wx_a = {}
    wh_a = {}
    for j in range(3):
        if j > 0:
            wx_a[j], bias_flags[f"bi{j}"] = _prep_weight(
                np.asarray(inputs[f"Wx{j}"], np.float32),
                np.asarray(inputs[f"bi{j}"], np.float32),
            )
        wh_a[j], bias_flags[f"br{j}"] = _prep_weight(
            np.asarray(inputs[f"Wh{j}"], np.float32),
            np.asarray(inputs[f"br{j}"], np.float32),
        )
    wd_p = wd_m.reshape(6, 128, D)
    bias_flags["bd"] = float(np.abs(bd).max()) > 0.0
    if bias_flags["bd"]:
        bc = np.zeros((1, 128, D), np.float32)
        bc[0, 0, :] = bd
        wd_p = np.concatenate([wd_p, bc], axis=0)
    wd_a = wd_p.astype(F16).reshape(-1, D)

    # warmup inputs, transposed per step: [T, D, B] -> [T*4*128, B]
    xt_a = np.ascontiguousarray(x.transpose(1, 2, 0)).reshape(n_warm * 512, B)
    xt_a = xt_a.astype(F16)

    # initial state, transposed: h^T chunk k at cols [k*64, (k+1)*64)
    h0f_l = []
    for j in range(3):
        ht = np.repeat(
            np.asarray(inputs[f"h0_{j}"], np.float32).reshape(U, 1), B, axis=1
        )  # [768, 64]
        h0f_l.append(ht.reshape(KU, 128, B).transpose(1, 0, 2).reshape(128, KU * B))
    h0f_a = np.concatenate(h0f_l, axis=0).astype(np.float32)

    ones_a = np.zeros((128, B), np.float32)
    ones_a[0, :] = 1.0
    ones_a = ones_a.astype(F16)

    nc = _build(n_warm, n_ar, bias_flags)
    in_map = {
        "wx0": wx0_a,
        "weff": weff_a,
        "wx1": wx_a[1],
        "wx2": wx_a[2],
        "wh0": wh_a[0],
        "wh1": wh_a[1],
        "wh2": wh_a[2],
        "wd": wd_a,
        "xt": xt_a,
        "h0f": h0f_a,
        "ones": ones_a,
    }
    res = run_bass_kernel_spmd(
        nc,
        [in_map],
        core_ids=[0],
        trace=os.environ.get("GRU_TRACE", "") == "1",
    )
    kernel._last = res
    kernel._last_nc = nc
    outT = np.asarray(res.results[0]["out"], np.float32)  # [(n_ar+1)*4*128, B]
    n_out = n_ar + 1
    return np.ascontiguousarray(
        outT.reshape(n_out, D, B).transpose(2, 0, 1)
    )


if __name__ == "__main__":
    print("smoke build only")


# revision 10
# speedup vs baseline: 2.0536x; 1.1049x over previous
"""Trainium2 Bass kernel for the 3-layer AR GRU (nn_AR_RNN_GRU).

Strategy
--------
The time recurrence is strictly sequential (127 dependent steps x 3 layers),
and cross-core exchange costs more than it saves (collective floor ~15us in
the cost model; remote SBUF-to-SBUF DMA is not available under this runtime).
The whole recurrence runs on ONE core with the full batch of 64.

This version uses a *transposed, weight-stationary* formulation: matmul cost
on the PE is set by the moving-operand column count, so we make the weight
block [128K x 128M] the stationary operand and stream the transposed
activations h^T (64 batch columns) through it:

    gates^T[m-chunk, batch] += W[k-chunk, m-chunk]^T-as-lhsT @ h^T[k-chunk, :]

 * Each [128,128] weight block costs only 64 moving columns (vs 128-equiv in
   the batch-stationary layout) -> ~2x less PE time for the same FLOPs.
 * The state lives as h^T tiles [128 unit-chunk, 64 batch] (fp16 for matmul,
   fp32 for the state update), so layer outputs are ALREADY in the rhs layout
   of the next matmuls: no PE transposes at all.
 * Gate psums are [128 gate-chunk, 64] slices packed into [128, 384] banks
   (Z, R, XH, HH x 2 parities = 8 PSUM banks); gate math runs on the full
   [128, 384] spans on ACT (sigmoid/tanh, and 1-z via scale=-1) and DVE.
 * The AR feedback folds dense+normalize into one effective matrix:
   gx0 = h2 @ (Wd @ (Wx0/std)) + beff, removing the dense+normalize from the
   critical path; the real prediction h2 @ Wd + bd streams off-path and is
   written transposed to DRAM (the host un-transposes at the end).
 * Per-slice gx emission order [r, z, xh] shortens the gate-math critical
   chain after the last matmul so the PE never stalls on the state update.
Biases are applied via an extra "ones" K-chunk whose lhsT row 0 holds the
bias vector -- only beff is nonzero in this problem.
"""

import os
import sys

import numpy as np

try:
    import concourse.bass as bass  # noqa: F401
except ImportError:  # grading env fallback
    sys.path.insert(0, "/opt/trn_rl_repo")

import concourse.bass as bass
import concourse.mybir as mybir
import concourse.tile as tile
from concourse import bacc
from concourse.bass_utils import run_bass_kernel_spmd

F16 = np.float16

B = 64  # batch
D = 512  # data dim
U = 768  # GRU units
G = 3 * U  # gate columns (z|r|h)
KU = U // 128  # 6 K-chunks for a 768-row operand
MG = G // 128  # 18 M-chunks of gate columns
MD = D // 128  # 4 M-chunks of data columns

T_IN = int(os.environ.get("GRU_TIN", "64"))
T_OUT = int(os.environ.get("GRU_TOUT", "64"))

# m-chunk emission order: r slices first (they start the gate-math chain),
# then z, then the candidate (xh/hh) slices.
M_ORDER = list(range(6, 12)) + list(range(0, 6)) + list(range(12, 18))


def _prep_weight(w, bias):
    """[K, G] fp32 (+bias [G]) -> ([n_chunks*128, G] fp16, has_bias)."""
    k = w.shape[0]
    assert k % 128 == 0
    wp = w.reshape(k // 128, 128, G)
    has_bias = bias is not None and float(np.abs(bias).max()) > 0.0
    if has_bias:
        bc = np.zeros((1, 128, G), np.float32)
        bc[0, 0, :] = bias
        wp = np.concatenate([wp, bc], axis=0)
    return wp.astype(F16).reshape(-1, G), has_bias


def _build(n_warm, n_ar, bias_flags):
    nc = bacc.Bacc(num_devices=1, name="gru_ar_t")
    f32, f16 = mybir.dt.float32, mybir.dt.float16
    n_steps = n_warm + n_ar
    n_out = n_ar + 1

    # ---- DRAM I/O ----
    wx0_c = 4 + bias_flags["bi0"]
    weff_c = 6 + bias_flags["beff"]
    wx0 = nc.dram_tensor("wx0", [wx0_c * 128, G], f16, kind="ExternalInput")
    weff = nc.dram_tensor("weff", [weff_c * 128, G], f16, kind="ExternalInput")
    wx_c = [wx0_c, 6 + bias_flags["bi1"], 6 + bias_flags["bi2"]]
    wx = [None] + [
        nc.dram_tensor(f"wx{j}", [wx_c[j] * 128, G], f16, kind="ExternalInput")
        for j in (1, 2)
    ]
    wh_c = [6 + bias_flags[f"br{j}"] for j in range(3)]
    wh = [
        nc.dram_tensor(f"wh{j}", [wh_c[j] * 128, G], f16, kind="ExternalInput")
        for j in range(3)
    ]
    wd_c = 6 + bias_flags["bd"]
    wd = nc.dram_tensor("wd", [wd_c * 128, D], f16, kind="ExternalInput")
    xt = nc.dram_tensor("xt", [n_warm * 4 * 128, B], f16, kind="ExternalInput")
    h0f = nc.dram_tensor("h0f", [3 * 128, KU * B], f32, kind="ExternalInput")
    ones = nc.dram_tensor("ones", [128, B], f16, kind="ExternalInput")
    # transposed output: step s, D-chunk m at rows (s*4+m)*128
    out = nc.dram_tensor("out", [n_out * 4 * 128, B], f32, kind="ExternalOutput")

    with tile.TileContext(nc) as tc:
        with (
            tc.tile_pool(name="wpool", bufs=1) as wpool,
            tc.tile_pool(name="state", bufs=1) as spool,
            tc.tile_pool(name="work", bufs=2) as work,
            tc.tile_pool(name="workb", bufs=2) as workb,
            tc.tile_pool(name="xs", bufs=3) as xs,
            tc.tile_pool(name="pz", bufs=2, space="PSUM") as pzp,
            tc.tile_pool(name="pr", bufs=2, space="PSUM") as prp,
            tc.tile_pool(name="pxh", bufs=2, space="PSUM") as pxhp,
            tc.tile_pool(name="phh", bufs=2, space="PSUM") as phhp,
        ):

            def load_w(dram, kc, ncols, tag):
                t = wpool.tile([128, kc * ncols], f16, tag=tag)
                for c in range(kc):
                    nc.sync.dma_start(
                        t[:, c * ncols : (c + 1) * ncols],
                        dram[c * 128 : (c + 1) * 128, :],
                    )
                return t

            # small constants + state first, then weights in first-use order
            ones_t = wpool.tile([128, B], f16, tag="ones")
            nc.sync.dma_start(ones_t[:], ones[:])
            hF = []  # fp32 transposed state [128, KU*64]
            hT = []  # fp16 copy for matmuls
            for j in range(3):
                f = spool.tile([128, KU * B], f32, tag=f"hF{j}")
                nc.sync.dma_start(f[:], h0f[j * 128 : (j + 1) * 128, :])
                hF.append(f)
                t = spool.tile([128, KU * B], f16, tag=f"hT{j}")
                hT.append(t)
                nc.vector.tensor_copy(t[:], f[:])

            wh_t = [load_w(wh[0], wh_c[0], G, "wh0")]
            # prefetch the first two warmup x tiles ahead of the bulk weights
            xpre = {}
            for tt in range(min(2, n_warm)):
                xtile = xs.tile([128, 4 * B], f16, tag="xt")
                for c in range(4):
                    nc.sync.dma_start(
                        xtile[:, c * B : (c + 1) * B],
                        xt[tt * 512 + c * 128 : tt * 512 + (c + 1) * 128, :],
                    )
                xpre[tt] = xtile
            nshare = max(wx0_c, weff_c)
            wshare_t = wpool.tile([128, nshare * G], f16, tag="wx0weff")
            for c in range(wx0_c):
                nc.sync.dma_start(
                    wshare_t[:, c * G : (c + 1) * G], wx0[c * 128 : (c + 1) * 128, :]
                )
            wx_t = [wshare_t]
            wh_t.append(load_w(wh[1], wh_c[1], G, "wh1"))
            wx_t.append(load_w(wx[1], wx_c[1], G, "wx1"))
            wh_t.append(load_w(wh[2], wh_c[2], G, "wh2"))
            wx_t.append(load_w(wx[2], wx_c[2], G, "wx2"))
            wd_t = load_w(wd, wd_c, D, "wd")

            sig = mybir.ActivationFunctionType.Sigmoid
            tanh = mybir.ActivationFunctionType.Tanh

            def gru_layer(j, gx_w, gx_kc, gx_rhs_fn):
                pz = pzp.tile([128, 6 * B], f32, tag="z")
                pr = prp.tile([128, 6 * B], f32, tag="r")
                pxh = pxhp.tile([128, 6 * B], f32, tag="xh")
                phh = phhp.tile([128, 6 * B], f32, tag="hh")

                def gh_rhs(k, _j=j):
                    return (
                        ones_t[:, 0:B] if k >= 6 else hT[_j][:, k * B : (k + 1) * B]
                    )

                # start=True clears the WHOLE psum bank, so exactly one
                # start (the first matmul into each tile) and one stop (the
                # last) per tile; interleaved slices rely on has_written
                # bits for overwrite-on-first-touch.
                plan = []  # (dst, m, w_t, k, rhs_fn)
                for k in range(wh_c[j]):
                    for m in M_ORDER:
                        dst = pz if m < 6 else (pr if m < 12 else phh)
                        plan.append((dst, m, wh_t[j], k, gh_rhs))
                # gx: bias chunk(s) and early k-chunks first (k-major), the
                # last 3 k-chunks m-major so pr (then pz, then pxh) complete
                # well before the stream end and the gate chain overlaps the
                # stream tail instead of following it
                real_ks = [k for k in range(gx_kc) if k < 6]
                bias_ks = [k for k in range(gx_kc) if k >= 6]
                n_late = min(3, len(real_ks))
                early_ks = bias_ks + real_ks[: len(real_ks) - n_late]
                late_ks = real_ks[len(real_ks) - n_late :]
                for k in early_ks:
                    for m in M_ORDER:
                        dst = pz if m < 6 else (pr if m < 12 else pxh)
                        plan.append((dst, m, gx_w, k, gx_rhs_fn))
                for m in M_ORDER:
                    for k in late_ks:
                        dst = pz if m < 6 else (pr if m < 12 else pxh)
                        plan.append((dst, m, gx_w, k, gx_rhs_fn))
                first_of = {}
                last_of = {}
                for i, (dst, *_rest) in enumerate(plan):
                    if id(dst) not in first_of:
                        first_of[id(dst)] = i
                    last_of[id(dst)] = i
                for i, (dst, m, w_t, k, rhs_fn) in enumerate(plan):
                    c = m % 6
                    nc.tensor.matmul(
                        dst[:, c * B : (c + 1) * B],
                        w_t[:, k * G + m * 128 : k * G + (m + 1) * 128],
                        rhs_fn(k),
                        start=first_of[id(dst)] == i,
                        stop=last_of[id(dst)] == i,
                        skip_group_check=True,
                    )

                # gate math on [128, 384] spans; the pre-work (r/z gates and
                # z*h) only needs pz/pr/phh, which complete before the xh
                # slices, so it overlaps the matmul stream tail.
                rs = work.tile([128, 6 * B], f32, tag="rs")
                nc.scalar.activation(rs[:], pr[:], sig)
                zs = work.tile([128, 6 * B], f32, tag="zs")
                nc.scalar.activation(zs[:], pz[:], sig)
                omz = work.tile([128, 6 * B], f32, tag="omz")
                nc.scalar.activation(omz[:], pz[:], sig, scale=-1.0)
                t1 = workb.tile([128, 6 * B], f32, tag="t1")
                nc.vector.tensor_mul(t1[:], rs[:], phh[:])
                zh = workb.tile([128, 6 * B], f32, tag="zh")
                nc.vector.tensor_mul(zh[:], zs[:], hF[j][:])
                # post-xh chain, split into two 192-col halves so the first
                # half of hT[j] is ready for the next gx stream early
                hc = workb.tile([128, 6 * B], f32, tag="hc")
                q = workb.tile([128, 6 * B], f32, tag="q")
                H = 3 * B
                for h0, h1 in ((0, H), (H, 2 * H)):
                    nc.vector.tensor_add(
                        t1[:, h0:h1], t1[:, h0:h1], pxh[:, h0:h1]
                    )
                    nc.scalar.activation(hc[:, h0:h1], t1[:, h0:h1], tanh)
                    nc.vector.tensor_mul(
                        q[:, h0:h1], omz[:, h0:h1], hc[:, h0:h1]
                    )
                    # fp16 state first: it is the next matmul's input
                    nc.vector.tensor_add(
                        hT[j][:, h0:h1], zh[:, h0:h1], q[:, h0:h1]
                    )
                # fp32 state update trails off the critical path
                nc.vector.tensor_add(hF[j][:], zh[:], q[:])

            for t in range(n_steps):
                warm = t < n_warm
                if warm:
                    if t in xpre:
                        xtile = xpre.pop(t)
                    else:
                        xtile = xs.tile([128, 4 * B], f16, tag="xt")
                        for c in range(4):
                            nc.sync.dma_start(
                                xtile[:, c * B : (c + 1) * B],
                                xt[t * 512 + c * 128 : t * 512 + (c + 1) * 128, :],
                            )

                    def gx0_rhs(k, _x=xtile):
                        return (
                            ones_t[:, 0:B] if k >= 4 else _x[:, k * B : (k + 1) * B]
                        )

                    gru_layer(0, wshare_t, wx0_c, gx0_rhs)
                    if t == n_warm - 1:
                        # overwrite the shared slot with Weff for the AR phase
                        for c in range(weff_c):
                            nc.sync.dma_start(
                                wshare_t[:, c * G : (c + 1) * G],
                                weff[c * 128 : (c + 1) * 128, :],
                            )
                else:

                    def gxar_rhs(k):
                        return (
                            ones_t[:, 0:B] if k >= 6 else hT[2][:, k * B : (k + 1) * B]
                        )

                    gru_layer(0, wshare_t, weff_c, gxar_rhs)

                # dense readout of h2(t-1): emitted after layer 0 of step t so
                # the PE never waits on the h2 state update
                if t >= n_warm:
                    pp = pxhp.tile([128, 6 * B], f32, tag="xh")
                    for k in range(wd_c):
                        rhs = (
                            ones_t[:, 0:B]
                            if k >= 6
                            else hT[2][:, k * B : (k + 1) * B]
                        )
                        for m in range(MD):
                            nc.tensor.matmul(
                                pp[:, m * B : (m + 1) * B],
                                wd_t[:, k * D + m * 128 : k * D + (m + 1) * 128],
                                rhs,
                                start=k == 0 and m == 0,
                                stop=k == wd_c - 1 and m == MD - 1,
                                skip_group_check=True,
                            )
                    prs = workb.tile([128, MD * B], f32, tag="pred")
                    nc.vector.tensor_copy(prs[:], pp[:, 0 : MD * B])
                    s = t - n_warm
                    for m in range(MD):
                        nc.sync.dma_start(
                            out[(s * 4 + m) * 128 : (s * 4 + m + 1) * 128, :],
                            prs[:, m * B : (m + 1) * B],
                        )

                for j in (1, 2):

                    def gx_rhs(k, _j=j):
                        return (
                            ones_t[:, 0:B]
                            if k >= 6
                            else hT[_j - 1][:, k * B : (k + 1) * B]
                        )

                    gru_layer(j, wx_t[j], wx_c[j], gx_rhs)

            # final prediction (from h2 of the last step)
            pp = pxhp.tile([128, 6 * B], f32, tag="xh")
            for k in range(wd_c):
                rhs = ones_t[:, 0:B] if k >= 6 else hT[2][:, k * B : (k + 1) * B]
                for m in range(MD):
                    nc.tensor.matmul(
                        pp[:, m * B : (m + 1) * B],
                        wd_t[:, k * D + m * 128 : k * D + (m + 1) * 128],
                        rhs,
                        start=k == 0 and m == 0,
                        stop=k == wd_c - 1 and m == MD - 1,
                        skip_group_check=True,
                    )
            prs = workb.tile([128, MD * B], f32, tag="pred")
            nc.vector.tensor_copy(prs[:], pp[:, 0 : MD * B])
            for m in range(MD):
                nc.sync.dma_start(
                    out[(n_ar * 4 + m) * 128 : (n_ar * 4 + m + 1) * 128, :],
                    prs[:, m * B : (m + 1) * B],
                )
    nc.finalize()
    return nc


def kernel(**inputs):
    x = np.asarray(inputs["inputs"], np.float32)
    n_warm, n_ar = T_IN, T_OUT - 1
    x = x[:, :n_warm, :]

    mean = np.asarray(inputs["mean"], np.float32)[0]
    std = np.asarray(inputs["std"], np.float32)[0]
    wd_m = np.asarray(inputs["Wd"], np.float32)
    bd = np.asarray(inputs["bd"], np.float32)
    w1 = np.asarray(inputs["Wx0"], np.float32) / std[:, None]
    weff_m = wd_m @ w1
    beff = (bd - mean) @ w1 + np.asarray(inputs["bi0"], np.float32)

    bias_flags = {}
    wx0_a, bias_flags["bi0"] = _prep_weight(
        np.asarray(inputs["Wx0"], np.float32), np.asarray(inputs["bi0"], np.float32)
    )
    weff_a, bias_flags["beff"] = _prep_weight(weff_m, beff)
    wx_a = {}
    wh_a = {}
    for j in range(3):
        if j > 0:
            wx_a[j], bias_flags[f"bi{j}"] = _prep_weight(
                np.asarray(inputs[f"Wx{j}"], np.float32),
                np.asarray(inputs[f"bi{j}"], np.float32),
            )
        wh_a[j], bias_flags[f"br{j}"] = _prep_weight(
            np.asarray(inputs[f"Wh{j}"], np.float32),
            np.asarray(inputs[f"br{j}"], np.float32),
        )
    wd_p = wd_m.reshape(6, 128, D)
    bias_flags["bd"] = float(np.abs(bd).max()) > 0.0
    if bias_flags["bd"]:
        bc = np.zeros((1, 128, D), np.float32)
        bc[0, 0, :] = bd
        wd_p = np.concatenate([wd_p, bc], axis=0)
    wd_a = wd_p.astype(F16).reshape(-1, D)

    # warmup inputs, transposed per step: [T, D, B] -> [T*4*128, B]
    xt_a = np.ascontiguousarray(x.transpose(1, 2, 0)).reshape(n_warm * 512, B)
    xt_a = xt_a.astype(F16)

    # initial state, transposed: h^T chunk k at cols [k*64, (k+1)*64)
    h0f_l = []
    for j in range(3):
        ht = np.repeat(
            np.asarray(inputs[f"h0_{j}"], np.float32).reshape(U, 1), B, axis=1
        )  # [768, 64]
        h0f_l.append(ht.reshape(KU, 128, B).transpose(1, 0, 2).reshape(128, KU * B))
    h0f_a = np.concatenate(h0f_l, axis=0).astype(np.float32)

    ones_a = np.zeros((128, B), np.float32)
    ones_a[0, :] = 1.0
    ones_a = ones_a.astype(F16)

    nc = _build(n_warm, n_ar, bias_flags)
    in_map = {
        "wx0": wx0_a,
        "weff": weff_a,
        "wx1": wx_a[1],
        "wx2": wx_a[2],
        "wh0": wh_a[0],
        "wh1": wh_a[1],
        "wh2": wh_a[2],
        "wd": wd_a,
        "xt": xt_a,
        "h0f": h0f_a,
        "ones": ones_a,
    }
    res = run_bass_kernel_spmd(
        nc,
        [in_map],
        core_ids=[0],
        trace=os.environ.get("GRU_TRACE", "") == "1",
    )
    kernel._last = res
    kernel._last_nc = nc
    outT = np.asarray(res.results[0]["out"], np.float32)  # [(n_ar+1)*4*128, B]
    n_out = n_ar + 1
    return np.ascontiguousarray(
        outT.reshape(n_out, D, B).transpose(2, 0, 1)
    )


if __name__ == "__main__":
    print("smoke build only")


# revision 41
# speedup vs baseline: 2.0582x; 1.0022x over previous
"""Trainium2 Bass kernel for the 3-layer AR GRU (nn_AR_RNN_GRU).

Strategy
--------
The time recurrence is strictly sequential (127 dependent steps x 3 layers),
and cross-core exchange costs more than it saves (collective floor ~15us in
the cost model; remote SBUF-to-SBUF DMA is not available under this runtime).
The whole recurrence runs on ONE core with the full batch of 64.

This version uses a *transposed, weight-stationary* formulation: matmul cost
on the PE is set by the moving-operand column count, so we make the weight
block [128K x 128M] the stationary operand and stream the transposed
activations h^T (64 batch columns) through it:

    gates^T[m-chunk, batch] += W[k-chunk, m-chunk]^T-as-lhsT @ h^T[k-chunk, :]

 * Each [128,128] weight block costs only 64 moving columns (vs 128-equiv in
   the batch-stationary layout) -> ~2x less PE time for the same FLOPs.
 * The state lives as h^T tiles [128 unit-chunk, 64 batch] (fp16 for matmul,
   fp32 for the state update), so layer outputs are ALREADY in the rhs layout
   of the next matmuls: no PE transposes at all.
 * Gate psums are [128 gate-chunk, 64] slices packed into [128, 384] banks
   (Z, R, XH, HH x 2 parities = 8 PSUM banks); gate math runs on the full
   [128, 384] spans on ACT (sigmoid/tanh, and 1-z via scale=-1) and DVE.
 * The AR feedback folds dense+normalize into one effective matrix:
   gx0 = h2 @ (Wd @ (Wx0/std)) + beff, removing the dense+normalize from the
   critical path; the real prediction h2 @ Wd + bd streams off-path and is
   written transposed to DRAM (the host un-transposes at the end).
 * Per-slice gx emission order [r, z, xh] shortens the gate-math critical
   chain after the last matmul so the PE never stalls on the state update.
Biases are applied via an extra "ones" K-chunk whose lhsT row 0 holds the
bias vector -- only beff is nonzero in this problem.
"""

import os
import sys

import numpy as np

try:
    import concourse.bass as bass  # noqa: F401
except ImportError:  # grading env fallback
    sys.path.insert(0, "/opt/trn_rl_repo")

import concourse.bass as bass
import concourse.mybir as mybir
import concourse.tile as tile
from concourse import bacc
from concourse.bass_utils import run_bass_kernel_spmd

F16 = np.float16

B = 64  # batch
D = 512  # data dim
U = 768  # GRU units
G = 3 * U  # gate columns (z|r|h)
KU = U // 128  # 6 K-chunks for a 768-row operand
MG = G // 128  # 18 M-chunks of gate columns
MD = D // 128  # 4 M-chunks of data columns

T_IN = int(os.environ.get("GRU_TIN", "64"))
T_OUT = int(os.environ.get("GRU_TOUT", "64"))

# m-chunk emission order: r slices first (they start the gate-math chain),
# then z, then the candidate (xh/hh) slices.
M_ORDER = list(range(6, 12)) + list(range(0, 6)) + list(range(12, 18))


def _prep_weight(w, bias):
    """[K, G] fp32 (+bias [G]) -> ([n_chunks*128, G] fp16, has_bias)."""
    k = w.shape[0]
    assert k % 128 == 0
    wp = w.reshape(k // 128, 128, G)
    has_bias = bias is not None and float(np.abs(bias).max()) > 0.0
    if has_bias:
        bc = np.zeros((1, 128, G), np.float32)
        bc[0, 0, :] = bias
        wp = np.concatenate([wp, bc], axis=0)
    return wp.astype(F16).reshape(-1, G), has_bias


def _build(n_warm, n_ar, bias_flags):
    nc = bacc.Bacc(num_devices=1, name="gru_ar_t")
    f32, f16 = mybir.dt.float32, mybir.dt.float16
    n_steps = n_warm + n_ar
    n_out = n_ar + 1

    # ---- DRAM I/O ----
    wx0_c = 4 + bias_flags["bi0"]
    weff_c = 6 + bias_flags["beff"]
    wx0 = nc.dram_tensor("wx0", [wx0_c * 128, G], f16, kind="ExternalInput")
    weff = nc.dram_tensor("weff", [weff_c * 128, G], f16, kind="ExternalInput")
    wx_c = [wx0_c, 6 + bias_flags["bi1"], 6 + bias_flags["bi2"]]
    wx = [None] + [
        nc.dram_tensor(f"wx{j}", [wx_c[j] * 128, G], f16, kind="ExternalInput")
        for j in (1, 2)
    ]
    wh_c = [6 + bias_flags[f"br{j}"] for j in range(3)]
    wh = [
        nc.dram_tensor(f"wh{j}", [wh_c[j] * 128, G], f16, kind="ExternalInput")
        for j in range(3)
    ]
    wd_c = 6 + bias_flags["bd"]
    wd = nc.dram_tensor("wd", [wd_c * 128, D], f16, kind="ExternalInput")
    # beff per-partition bias columns: sections (z, r, xh, -z), chunk c at col c
    beff_d = nc.dram_tensor("beff", [4 * 128, KU], f32, kind="ExternalInput")
    xt = nc.dram_tensor("xt", [n_warm * 4 * 128, B], f16, kind="ExternalInput")
    h0f = nc.dram_tensor("h0f", [3 * 128, KU * B], f32, kind="ExternalInput")
    ones = nc.dram_tensor("ones", [128, B], f16, kind="ExternalInput")
    # transposed output: step s, D-chunk m at rows (s*4+m)*128
    out = nc.dram_tensor("out", [n_out * 4 * 128, B], f32, kind="ExternalOutput")

    with tile.TileContext(nc) as tc:
        with (
            tc.tile_pool(name="wpool", bufs=1) as wpool,
            tc.tile_pool(name="state", bufs=1) as spool,
            tc.tile_pool(name="work", bufs=2) as work,
            tc.tile_pool(name="workb", bufs=2) as workb,
            tc.tile_pool(name="xs", bufs=3) as xs,
            tc.tile_pool(name="pz", bufs=2, space="PSUM") as pzp,
            tc.tile_pool(name="pr", bufs=2, space="PSUM") as prp,
            tc.tile_pool(name="pxh", bufs=2, space="PSUM") as pxhp,
            tc.tile_pool(name="phh", bufs=2, space="PSUM") as phhp,
        ):

            def load_w(dram, kc, ncols, tag):
                t = wpool.tile([128, kc * ncols], f16, tag=tag)
                for c in range(kc):
                    nc.sync.dma_start(
                        t[:, c * ncols : (c + 1) * ncols],
                        dram[c * 128 : (c + 1) * 128, :],
                    )
                return t

            # small constants + state first, then weights in first-use order
            ones_t = wpool.tile([128, B], f16, tag="ones")
            nc.sync.dma_start(ones_t[:], ones[:])
            beff_t = []
            for g in range(4):
                bt = wpool.tile([128, KU], f32, tag=f"beff{g}")
                nc.sync.dma_start(bt[:], beff_d[g * 128 : (g + 1) * 128, :])
                beff_t.append(bt)
            hF = []  # fp32 transposed state [128, KU*64]
            hT = []  # fp16 copy for matmuls
            for j in range(3):
                f = spool.tile([128, KU * B], f32, tag=f"hF{j}")
                nc.sync.dma_start(f[:], h0f[j * 128 : (j + 1) * 128, :])
                hF.append(f)
                t = spool.tile([128, KU * B], f16, tag=f"hT{j}")
                hT.append(t)
                nc.vector.tensor_copy(t[:], f[:])

            wh_t = [load_w(wh[0], wh_c[0], G, "wh0")]
            # prefetch the first two warmup x tiles ahead of the bulk weights
            S = min(3, n_warm)  # layer-skewed warm prefix length
            xpre = {}
            for tt in range(min(max(2, S), n_warm)):
                xtile = xs.tile([128, 4 * B], f16, tag="xt")
                for c in range(4):
                    nc.sync.dma_start(
                        xtile[:, c * B : (c + 1) * B],
                        xt[tt * 512 + c * 128 : tt * 512 + (c + 1) * 128, :],
                    )
                xpre[tt] = xtile
            nshare = max(wx0_c, weff_c)
            wshare_t = wpool.tile([128, nshare * G], f16, tag="wx0weff")
            for c in range(wx0_c):
                nc.sync.dma_start(
                    wshare_t[:, c * G : (c + 1) * G], wx0[c * 128 : (c + 1) * 128, :]
                )
            wx_t = [wshare_t]
            wh_t.append(load_w(wh[1], wh_c[1], G, "wh1"))
            wx_t.append(load_w(wx[1], wx_c[1], G, "wx1"))
            wh_t.append(load_w(wh[2], wh_c[2], G, "wh2"))
            wx_t.append(load_w(wx[2], wx_c[2], G, "wx2"))
            wd_t = load_w(wd, wd_c, D, "wd")

            sig = mybir.ActivationFunctionType.Sigmoid
            tanh = mybir.ActivationFunctionType.Tanh

            def gru_layer(
                j, gx_w, gx_kc, gx_rhs_fn, mid_pe=None, bias=False,
                gh_src=None, out16=None,
            ):
                if gh_src is None:
                    gh_src = hT[j]
                if out16 is None:
                    out16 = hT[j]
                pz = pzp.tile([128, 6 * B], f32, tag="z")
                pr = prp.tile([128, 6 * B], f32, tag="r")
                pxh = pxhp.tile([128, 6 * B], f32, tag="xh")
                phh = phhp.tile([128, 6 * B], f32, tag="hh")

                def gh_rhs(k, _s=gh_src):
                    return (
                        ones_t[:, 0:B] if k >= 6 else _s[:, k * B : (k + 1) * B]
                    )

                # start=True clears the WHOLE psum bank, so exactly one
                # start (the first matmul into each tile) and one stop (the
                # last) per tile; interleaved slices rely on has_written
                # bits for overwrite-on-first-touch.
                plan = []  # (dst, m, w_t, k, rhs_fn)
                for k in range(wh_c[j]):
                    for m in M_ORDER:
                        dst = pz if m < 6 else (pr if m < 12 else phh)
                        plan.append((dst, m, wh_t[j], k, gh_rhs))
                # gx: bias chunk(s) and early k-chunks first (k-major), the
                # last 3 k-chunks m-major so pr (then pz, then pxh) complete
                # well before the stream end and the gate chain overlaps the
                # stream tail instead of following it
                real_ks = [k for k in range(gx_kc) if k < 6]
                bias_ks = [k for k in range(gx_kc) if k >= 6]
                n_late = min(3, len(real_ks))
                early_ks = bias_ks + real_ks[: len(real_ks) - n_late]
                late_ks = real_ks[len(real_ks) - n_late :]
                for k in early_ks:
                    for m in M_ORDER:
                        dst = pz if m < 6 else (pr if m < 12 else pxh)
                        plan.append((dst, m, gx_w, k, gx_rhs_fn))
                for m in M_ORDER:
                    for k in late_ks:
                        dst = pz if m < 6 else (pr if m < 12 else pxh)
                        plan.append((dst, m, gx_w, k, gx_rhs_fn))
                n_gh = wh_c[j] * len(M_ORDER)
                first_of = {}
                last_of = {}
                for i, (dst, *_rest) in enumerate(plan):
                    if id(dst) not in first_of:
                        first_of[id(dst)] = i
                    last_of[id(dst)] = i
                for i, (dst, m, w_t, k, rhs_fn) in enumerate(plan):
                    if i == n_gh and mid_pe is not None:
                        mid_pe()
                    c = m % 6
                    nc.tensor.matmul(
                        dst[:, c * B : (c + 1) * B],
                        w_t[:, k * G + m * 128 : k * G + (m + 1) * 128],
                        rhs_fn(k),
                        start=first_of[id(dst)] == i,
                        stop=last_of[id(dst)] == i,
                        skip_group_check=True,
                    )

                # gate math on [128, 384] spans; the pre-work (r/z gates and
                # z*h) only needs pz/pr/phh, which complete before the xh
                # slices, so it overlaps the matmul stream tail.
                # beff (AR layer 0) rides in via the ACT bias operand
                # (per-partition [128,1]); per-chunk calls keep it off the
                # DVE critical chain entirely
                def act(dst, src, func, sec, lo, hi, scale=1.0):
                    if bias:
                        for c in range(lo, hi):
                            cs = slice(c * B, (c + 1) * B)
                            nc.scalar.activation(
                                dst[:, cs],
                                src[:, cs],
                                func,
                                bias=beff_t[sec][:, c : c + 1],
                                scale=scale,
                            )
                    else:
                        cs = slice(lo * B, hi * B)
                        nc.scalar.activation(dst[:, cs], src[:, cs], func, scale=scale)

                rs = work.tile([128, 6 * B], f32, tag="rs")
                act(rs, pr, sig, 1, 0, KU)
                zs = work.tile([128, 6 * B], f32, tag="zs")
                act(zs, pz, sig, 0, 0, KU)
                omz = work.tile([128, 6 * B], f32, tag="omz")
                act(omz, pz, sig, 3, 0, KU, scale=-1.0)
                t1 = workb.tile([128, 6 * B], f32, tag="t1")
                nc.vector.tensor_mul(t1[:], rs[:], phh[:])
                zh = workb.tile([128, 6 * B], f32, tag="zh")
                nc.vector.tensor_mul(zh[:], zs[:], hF[j][:])
                # post-xh chain, split into two 192-col halves so the first
                # half of hT[j] is ready for the next gx stream early
                # (tanh runs in place on t1 to save SBUF)
                hc = t1
                q = workb.tile([128, 6 * B], f32, tag="q")
                for hi in (0, 1):
                    h0, h1 = hi * 3 * B, (hi + 1) * 3 * B
                    g = slice(h0, h1)
                    nc.vector.tensor_add(t1[:, g], t1[:, g], pxh[:, g])
                    act(hc, t1, tanh, 2, 3 * hi, 3 * hi + 3)
                    nc.vector.tensor_mul(q[:, g], omz[:, g], hc[:, g])
                    # fp16 state first: it is the next matmul's input
                    nc.vector.tensor_add(out16[:, g], zh[:, g], q[:, g])
                # fp32 state update trails off the critical path, on the
                # otherwise-idle Pool engine so it never delays DVE
                nc.gpsimd.tensor_add(hF[j][:], zh[:], q[:])

            # ---- layer-skewed warm prefix: run layer j for steps 0..S-1
            # before layer j+1 touches step 0, so early compute only needs
            # the layer-j weights while the rest are still streaming in ----
            sc = {}
            for j in range(3):
                sc[j] = []
                for tt in range(S - 1):
                    sc_t = spool.tile(
                        [128, KU * B], f16, tag=f"sc{j}_{tt}", name=f"sc{j}_{tt}"
                    )
                    sc[j].append(sc_t)
            for j in range(3):
                for t in range(S):
                    out16 = hT[j] if t == S - 1 else sc[j][t]
                    gh_src = hT[j] if t == 0 else sc[j][t - 1]
                    if j == 0:
                        xtile = xpre.pop(t)

                        def gx_rhs(k, _x=xtile):
                            return (
                                ones_t[:, 0:B]
                                if k >= 4
                                else _x[:, k * B : (k + 1) * B]
                            )

                        gru_layer(
                            0, wshare_t, wx0_c, gx_rhs,
                            gh_src=gh_src, out16=out16,
                        )
                    else:
                        src_prev = hT[j - 1] if t == S - 1 else sc[j - 1][t]

                        def gx_rhs(k, _s=src_prev):
                            return (
                                ones_t[:, 0:B]
                                if k >= 6
                                else _s[:, k * B : (k + 1) * B]
                            )

                        gru_layer(
                            j, wx_t[j], wx_c[j], gx_rhs,
                            gh_src=gh_src, out16=out16,
                        )
                if j == 0 and S == n_warm:
                    for c in range(weff_c):
                        nc.sync.dma_start(
                            wshare_t[:, c * G : (c + 1) * G],
                            weff[c * 128 : (c + 1) * 128, :],
                        )

            for t in range(S, n_steps):
                warm = t < n_warm
                if warm:
                    if t in xpre:
                        xtile = xpre.pop(t)
                    else:
                        xtile = xs.tile([128, 4 * B], f16, tag="xt")
                        for c in range(4):
                            nc.sync.dma_start(
                                xtile[:, c * B : (c + 1) * B],
                                xt[t * 512 + c * 128 : t * 512 + (c + 1) * 128, :],
                            )

                    def gx0_rhs(k, _x=xtile):
                        return (
                            ones_t[:, 0:B] if k >= 4 else _x[:, k * B : (k + 1) * B]
                        )

                    gru_layer(0, wshare_t, wx0_c, gx0_rhs)
                    if t == n_warm - 1:
                        # overwrite the shared slot with Weff for the AR phase
                        for c in range(weff_c):
                            nc.sync.dma_start(
                                wshare_t[:, c * G : (c + 1) * G],
                                weff[c * 128 : (c + 1) * 128, :],
                            )
                else:

                    def gxar_rhs(k):
                        return (
                            ones_t[:, 0:B] if k >= 6 else hT[2][:, k * B : (k + 1) * B]
                        )

                    gru_layer(0, wshare_t, weff_c, gxar_rhs)

                def gx_rhs1(k):
                    return (
                        ones_t[:, 0:B] if k >= 6 else hT[0][:, k * B : (k + 1) * B]
                    )

                # dense readout of h2(t-1): split into two halves, one after
                # gh1 and one after gh2, so each half covers the PE wait on
                # the previous layer's state update (hT[2] is still h2(t-1)
                # until layer 2 runs)
                if t >= n_warm:
                    pp = pxhp.tile([128, 6 * B], f32, tag="xh")
                    ks_a = list(range(3))
                    ks_b = list(range(3, wd_c))
                else:
                    pp = None
                    ks_a = ks_b = []

                def dense_part(ks):
                    for k in ks:
                        rhs = (
                            ones_t[:, 0:B]
                            if k >= 6
                            else hT[2][:, k * B : (k + 1) * B]
                        )
                        for m in range(MD):
                            nc.tensor.matmul(
                                pp[:, m * B : (m + 1) * B],
                                wd_t[:, k * D + m * 128 : k * D + (m + 1) * 128],
                                rhs,
                                start=k == 0 and m == 0,
                                stop=k == wd_c - 1 and m == MD - 1,
                                skip_group_check=True,
                            )

                gru_layer(1, wx_t[1], wx_c[1], gx_rhs1, mid_pe=lambda: dense_part(ks_a))

                def gx_rhs2(k):
                    return (
                        ones_t[:, 0:B] if k >= 6 else hT[1][:, k * B : (k + 1) * B]
                    )

                gru_layer(2, wx_t[2], wx_c[2], gx_rhs2, mid_pe=lambda: dense_part(ks_b))

                if t >= n_warm:
                    prs = workb.tile([128, MD * B], f32, tag="pred")
                    nc.vector.tensor_copy(prs[:], pp[:, 0 : MD * B])
                    s = t - n_warm
                    for m in range(MD):
                        nc.sync.dma_start(
                            out[(s * 4 + m) * 128 : (s * 4 + m + 1) * 128, :],
                            prs[:, m * B : (m + 1) * B],
                        )

            # final prediction (from h2 of the last step)
            pp = pxhp.tile([128, 6 * B], f32, tag="xh")
            for k in range(wd_c):
                rhs = ones_t[:, 0:B] if k >= 6 else hT[2][:, k * B : (k + 1) * B]
                for m in range(MD):
                    nc.tensor.matmul(
                        pp[:, m * B : (m + 1) * B],
                        wd_t[:, k * D + m * 128 : k * D + (m + 1) * 128],
                        rhs,
                        start=k == 0 and m == 0,
                        stop=k == wd_c - 1 and m == MD - 1,
                        skip_group_check=True,
                    )
            prs = workb.tile([128, MD * B], f32, tag="pred")
            nc.vector.tensor_copy(prs[:], pp[:, 0 : MD * B])
            for m in range(MD):
                nc.sync.dma_start(
                    out[(n_ar * 4 + m) * 128 : (n_ar * 4 + m + 1) * 128, :],
                    prs[:, m * B : (m + 1) * B],
                )
    nc.finalize()
    return nc


def kernel(**inputs):
    x = np.asarray(inputs["inputs"], np.float32)
    n_warm, n_ar = T_IN, T_OUT - 1
    x = x[:, :n_warm, :]

    mean = np.asarray(inputs["mean"], np.float32)[0]
    std = np.asarray(inputs["std"], np.float32)[0]
    wd_m = np.asarray(inputs["Wd"], np.float32)
    bd = np.asarray(inputs["bd"], np.float32)
    w1 = np.asarray(inputs["Wx0"], np.float32) / std[:, None]
    weff_m = wd_m @ w1
    beff = (bd - mean) @ w1 + np.asarray(inputs["bi0"], np.float32)

    bias_flags = {}
    wx0_a, bias_flags["bi0"] = _prep_weight(
        np.asarray(inputs["Wx0"], np.float32), np.asarray(inputs["bi0"], np.float32)
    )
    weff_a, bias_flags["beff"] = _prep_weight(weff_m, beff)
    # dormant ACT-bias path (bias=False everywhere); zeros keep the input bound
    beff_a = np.zeros((4 * 128, KU), np.float32)
    wx_a = {}
    wh_a = {}
    for j in range(3):
        if j > 0:
            wx_a[j], bias_flags[f"bi{j}"] = _prep_weight(
                np.asarray(inputs[f"Wx{j}"], np.float32),
                np.asarray(inputs[f"bi{j}"], np.float32),
            )
        wh_a[j], bias_flags[f"br{j}"] = _prep_weight(
            np.asarray(inputs[f"Wh{j}"], np.float32),
            np.asarray(inputs[f"br{j}"], np.float32),
        )
    wd_p = wd_m.reshape(6, 128, D)
    bias_flags["bd"] = float(np.abs(bd).max()) > 0.0
    if bias_flags["bd"]:
        bc = np.zeros((1, 128, D), np.float32)
        bc[0, 0, :] = bd
        wd_p = np.concatenate([wd_p, bc], axis=0)
    wd_a = wd_p.astype(F16).reshape(-1, D)

    # warmup inputs, transposed per step: [T, D, B] -> [T*4*128, B]
    xt_a = np.ascontiguousarray(x.transpose(1, 2, 0)).reshape(n_warm * 512, B)
    xt_a = xt_a.astype(F16)

    # initial state, transposed: h^T chunk k at cols [k*64, (k+1)*64)
    h0f_l = []
    for j in range(3):
        ht = np.repeat(
            np.asarray(inputs[f"h0_{j}"], np.float32).reshape(U, 1), B, axis=1
        )  # [768, 64]
        h0f_l.append(ht.reshape(KU, 128, B).transpose(1, 0, 2).reshape(128, KU * B))
    h0f_a = np.concatenate(h0f_l, axis=0).astype(np.float32)

    ones_a = np.zeros((128, B), np.float32)
    ones_a[0, :] = 1.0
    ones_a = ones_a.astype(F16)

    nc = _build(n_warm, n_ar, bias_flags)
    in_map = {
        "wx0": wx0_a,
        "weff": weff_a,
        "wx1": wx_a[1],
        "wx2": wx_a[2],
        "wh0": wh_a[0],
        "wh1": wh_a[1],
        "wh2": wh_a[2],
        "wd": wd_a,
        "xt": xt_a,
        "h0f": h0f_a,
        "ones": ones_a,
        "beff": beff_a,
    }
    res = run_bass_kernel_spmd(
        nc,
        [in_map],
        core_ids=[0],
        trace=os.environ.get("GRU_TRACE", "") == "1",
    )
    kernel._last = res
    kernel._last_nc = nc
    outT = np.asarray(res.results[0]["out"], np.float32)  # [(n_ar+1)*4*128, B]
    n_out = n_ar + 1
    return np.ascontiguousarray(
        outT.reshape(n_out, D, B).transpose(2, 0, 1)
    )


if __name__ == "__main__":
    print("smoke build only")
